# revision 6
# baseline (speedup 1.0000x reference)
"""GraphSAGE 2-layer minibatch kernel for 8 TRN2 NeuronCores.

Strategy: data-parallel over the 1024-target batch (128 targets/core).
The host lays out each core's working set as fp16 DRAM streams in
compute order, pre-transposed to feature-major: per block (block 0 =
targets, blocks 1..10 = the s2-major nb2 rows) a self tile
[128 feat-partitions, 2 feat-chunks x 128 rows] and a neighbor slab
[128, 2 chunks x 128 rows x 25 slots] with the slot axis innermost so
the 25-neighbor mean folds in a single contiguous DVE tensor_reduce
per block (mean scale folded into pre-scaled fp16 weights). The device
streams the slabs with plain contiguous DMA split across both HWDGE
queues (sync + scalar), and runs the SAGE layer per block: fp16 matmul
with f32 PSUM accumulate + bias/relu + feature-major L2 normalize (PE
ones-matmul column reduce, ACT sqrt, DVE fast reciprocal, PE
broadcast). Layer 2 consumes the feature-major layer-1 outputs
directly (block 0 = self half, running sum of blocks 1..10 = agg
half). All DMAs are issued up-front (the full ~19MB/core stream fits
in SBUF) so the kernel runs at HBM streaming bandwidth with compute
chasing the stream.
"""

import numpy as np

N_NODES = 100000
D = 256
H = 256
B = 1024
S1 = 25
S2 = 10
NCORES = 8
BL = B // NCORES          # 128 rows per core
NBLK = 1 + S2             # 11 blocks of 128 layer-1 rows per core
P = 128
CH = D // P               # 2 feature chunks
NBW = CH * P * S1         # 6400 neighbor cols per block
KC = 4                    # contraction chunks per layer (2*D/P)
HC = H // P               # 2 output-feature chunks

_PROG = None  # cached so repeat calls reuse the built program


def _build_program():
    import concourse.mybir as mybir
    from concourse.bacc import Bacc
    from concourse.tile import TileContext

    f32 = mybir.dt.float32
    f16 = mybir.dt.float16
    AF = mybir.ActivationFunctionType
    add_op = mybir.AluOpType.add
    mult_op = mybir.AluOpType.mult
    AX = mybir.AxisListType

    nc = Bacc(trn_type="TRN2")

    self_d = nc.dram_tensor("selfs", (P, NBLK * CH * P), f16,
                            kind="ExternalInput")
    nb_d = nc.dram_tensor("nb", (NBLK * P, NBW), f16, kind="ExternalInput")
    w1c_d = nc.dram_tensor("w1c", (P, KC * H), f16, kind="ExternalInput")
    w2c_d = nc.dram_tensor("w2c", (P, KC * H), f16, kind="ExternalInput")
    b1c_d = nc.dram_tensor("b1c", (P, HC), f32, kind="ExternalInput")
    b2c_d = nc.dram_tensor("b2c", (P, HC), f32, kind="ExternalInput")
    zT_d = nc.dram_tensor("zT", (H, P), f32, kind="ExternalOutput")

    with TileContext(nc) as tc:
        with (
            tc.tile_pool(name="const", bufs=1) as cpool,
            tc.tile_pool(name="nbs", bufs=NBLK) as nbpool,
            tc.tile_pool(name="agg", bufs=2) as apool,
            tc.tile_pool(name="zsb", bufs=2) as zpool,
            tc.tile_pool(name="sq", bufs=2) as sqpool,
            tc.tile_pool(name="nrm", bufs=2) as nrmpool,
            tc.tile_pool(name="hn", bufs=2) as hnpool,
            tc.tile_pool(name="h1", bufs=1) as h1pool,
            tc.tile_pool(name="mm_ps", bufs=2, space="PSUM") as mmpool,
            tc.tile_pool(name="ss_ps", bufs=2, space="PSUM") as sspool,
            tc.tile_pool(name="bc_ps", bufs=2, space="PSUM") as bcpool,
        ):
            # ---- constants (sync queue, ahead of the stream) ---------------
            w1_sb = cpool.tile([P, KC * H], f16, tag="w1")
            nc.sync.dma_start(out=w1_sb[:], in_=w1c_d[:])
            w2_sb = cpool.tile([P, KC * H], f16, tag="w2")
            nc.sync.dma_start(out=w2_sb[:], in_=w2c_d[:])
            b1_sb = cpool.tile([P, HC], f32, tag="b1")
            nc.sync.dma_start(out=b1_sb[:], in_=b1c_d[:])
            b2_sb = cpool.tile([P, HC], f32, tag="b2")
            nc.sync.dma_start(out=b2_sb[:], in_=b2c_d[:])
            selfs_sb = cpool.tile([P, NBLK * CH * P], f16, tag="selfs")
            nc.sync.dma_start(out=selfs_sb[:], in_=self_d[:])

            ones16 = cpool.tile([P, 1], f16, tag="ones16")
            nc.gpsimd.memset(ones16[:], 1.0)
            ones32 = cpool.tile([1, P], f32, tag="ones32")
            nc.gpsimd.memset(ones32[:], 1.0)
            eps_sb = cpool.tile([1, 1], f32, tag="eps")
            nc.gpsimd.memset(eps_sb[:], 1e-8)

            # ---- stream: all neighbor-slab DMAs up-front, two queues -------
            slabs = []
            for i in range(NBLK):
                src = (i + 1) % NBLK  # compute order 1..10 then 0
                t = nbpool.tile([P, NBW], f16, tag="nb")
                eng = nc.sync if i % 2 == 0 else nc.scalar
                eng.dma_start(out=t[:], in_=nb_d[src * P:(src + 1) * P, :])
                slabs.append((src, t))

            h1t_sb = h1pool.tile([P, H], f16, tag="h1t")     # block-0 result
            agg2_sb = h1pool.tile([P, H], f16, tag="agg2")   # sum blocks 1..10
            z2_sb = h1pool.tile([P, H], f32, tag="z2")

            def sage(cat_chunks, w_sb, b_sb, out_sb):
                """SAGE layer (matmul + bias/relu + column L2-normalize) on a
                feature-major batch tile of width P.

                cat_chunks: KC fp16 APs [P, P] (contraction chunks: features
                on partitions, batch columns on free dim).
                out_sb: [P, HC*P] AP; its dtype governs the store.
                """
                z_sb = zpool.tile([P, HC * P], f32, tag="z")
                for h in range(HC):
                    z_ps = mmpool.tile([P, P], f32, space="PSUM", tag="mm")
                    for k in range(KC):
                        nc.tensor.matmul(
                            out=z_ps[:],
                            lhsT=w_sb[:, k * H + h * P: k * H + (h + 1) * P],
                            rhs=cat_chunks[k],
                            start=(k == 0),
                            stop=(k == KC - 1),
                        )
                    nc.scalar.activation(
                        out=z_sb[:, h * P:(h + 1) * P],
                        in_=z_ps[:],
                        func=AF.Relu,
                        bias=b_sb[:, h:h + 1],
                    )
                # column sum of squares via PE (features are on partitions)
                sq_sb = sqpool.tile([P, HC * P], f16, tag="sq")
                nc.scalar.square(sq_sb[:], z_sb[:])
                ss_ps = sspool.tile([1, P], f32, space="PSUM", tag="ss")
                for h in range(HC):
                    nc.tensor.matmul(
                        out=ss_ps[:],
                        lhsT=ones16[:, 0:1],
                        rhs=sq_sb[:, h * P:(h + 1) * P],
                        start=(h == 0),
                        stop=(h == HC - 1),
                    )
                # n = sqrt(ssq + eps); eps keeps all-zero rows finite and
                # reciprocal_approx_fast away from its ±0/denorm edge cases
                # (z * 1/n = 0 * 1e4 = 0). 18-bit recip ≫ fp16 data noise.
                n_t = nrmpool.tile([1, P], f32, tag="nrm")
                nc.scalar.activation(n_t[:], ss_ps[:], AF.Sqrt, bias=eps_sb[:])
                inv = nrmpool.tile([1, P], f32, tag="inv")
                nc.vector.reciprocal_approx_fast(out=inv[:], in_=n_t[:])
                bc_ps = bcpool.tile([P, P], f32, space="PSUM", tag="bc")
                nc.tensor.matmul(
                    out=bc_ps[:], lhsT=ones32[0:1, :], rhs=inv[:],
                    start=True, stop=True,
                )
                for h in range(HC):
                    nc.vector.tensor_tensor(
                        out=out_sb[:, h * P:(h + 1) * P],
                        in0=z_sb[:, h * P:(h + 1) * P],
                        in1=bc_ps[:],
                        op=mult_op,
                    )

            # ---- layer 1: blocks 1..10 then block 0 ------------------------
            for src, slab_t in slabs:
                # fold 25 neighbor slots (innermost, contiguous) in one op
                agg_t = apool.tile([P, CH * P], f16, tag="agg")
                with nc.allow_low_precision(reason="fp16 neighbor-sum fold"):
                    nc.vector.tensor_reduce(
                        out=agg_t[:],
                        in_=slab_t[:].rearrange("p (q s) -> p q s", s=S1),
                        axis=AX.X,
                        op=add_op,
                    )
                cat = [
                    selfs_sb[:, (src * CH + 0) * P:(src * CH + 1) * P],
                    selfs_sb[:, (src * CH + 1) * P:(src * CH + 2) * P],
                    agg_t[:, 0:P],
                    agg_t[:, P:2 * P],
                ]
                if src == 0:
                    sage(cat, w1_sb, b1_sb, h1t_sb[:])
                elif src == 1:
                    # first neighbor block writes agg2 directly
                    sage(cat, w1_sb, b1_sb, agg2_sb[:])
                else:
                    hn_t = hnpool.tile([P, H], f16, tag="hn")
                    sage(cat, w1_sb, b1_sb, hn_t[:])
                    nc.vector.tensor_tensor(
                        out=agg2_sb[:], in0=agg2_sb[:], in1=hn_t[:],
                        op=add_op,
                    )

            # ---- layer 2 ---------------------------------------------------
            cat2 = [
                h1t_sb[:, 0:P], h1t_sb[:, P:2 * P],
                agg2_sb[:, 0:P], agg2_sb[:, P:2 * P],
            ]
            sage(cat2, w2_sb, b2_sb, z2_sb[:])
            for h in range(HC):
                nc.sync.dma_start(
                    out=zT_d[h * P:(h + 1) * P, :],
                    in_=z2_sb[:, h * P:(h + 1) * P],
                )

    nc.finalize()
    return nc


def _get_program():
    global _PROG
    if _PROG is None:
        _PROG = _build_program()
    return _PROG


def make_in_maps(x, targets, nb1_self, nb2, nb1_nb, W1, b1, W2, b2):
    """Host-side sharding/preprocessing -> per-core input dicts."""
    x = np.ascontiguousarray(np.asarray(x, dtype=np.float32))
    W1 = np.asarray(W1, dtype=np.float32)
    W2 = np.asarray(W2, dtype=np.float32)
    b1 = np.asarray(b1, dtype=np.float32)
    b2 = np.asarray(b2, dtype=np.float32)
    targets = np.asarray(targets).astype(np.int64)
    nb1_self = np.asarray(nb1_self).astype(np.int64)
    nb2 = np.asarray(nb2).astype(np.int64)
    nb1_nb = np.asarray(nb1_nb).astype(np.int64)

    # fold the neighbor-mean scale into the agg half of each weight matrix,
    # pre-chunked to the SBUF layout: w[p, k*H + m] = W.T[k*128 + p, m]
    def chunk_w(W, s):
        ws = np.concatenate([W[:, :D], W[:, D:] / s], axis=1)
        wt = ws.T.astype(np.float16)                 # [2D, H]
        return np.ascontiguousarray(
            wt.reshape(KC, P, H).transpose(1, 0, 2).reshape(P, KC * H)
        )

    w1c = chunk_w(W1, S1)
    w2c = chunk_w(W2, S2)
    b1c = np.ascontiguousarray(b1.reshape(HC, P).T)  # [P, HC]
    b2c = np.ascontiguousarray(b2.reshape(HC, P).T)

    in_maps = []
    for core in range(NCORES):
        sl = slice(core * BL, (core + 1) * BL)
        self_ids = np.empty((NBLK, BL), dtype=np.int64)
        nb_ids = np.empty((NBLK, BL, S1), dtype=np.int64)
        self_ids[0] = targets[sl]
        nb_ids[0] = nb1_self[sl]
        for j in range(S2):
            self_ids[1 + j] = nb2[sl][:, j]
            nb_ids[1 + j] = nb1_nb[sl][:, j, :]

        # selfs[p, (b*CH + c)*P + r] = x[self_ids[b, r], c*P + p]
        sarr = x[self_ids].astype(np.float16)        # [NBLK, BL, D]
        selfs = np.ascontiguousarray(
            sarr.reshape(NBLK, BL, CH, P)
                .transpose(3, 0, 2, 1)
                .reshape(P, NBLK * CH * P)
        )
        # nb[b*P + p, (c*P + r)*S1 + s] = x[nb_ids[b, r, s], c*P + p]
        narr = x[nb_ids].astype(np.float16)          # [NBLK, BL, S1, D]
        nb = np.ascontiguousarray(
            narr.reshape(NBLK, BL, S1, CH, P)
                .transpose(0, 4, 3, 1, 2)
                .reshape(NBLK * P, NBW)
        )
        in_maps.append({
            "selfs": selfs, "nb": nb,
            "w1c": w1c, "w2c": w2c, "b1c": b1c, "b2c": b2c,
        })
    return in_maps


def run(trace=False, **inputs):
    from concourse.bass_utils import run_bass_kernel_spmd

    nc = _get_program()
    in_maps = make_in_maps(**inputs)
    res = run_bass_kernel_spmd(
        nc, in_maps, core_ids=list(range(NCORES)), trace=trace
    )
    out = np.concatenate(
        [np.asarray(r["zT"]).T for r in res.results], axis=0
    ).astype(np.float32)
    return out, res


def kernel(**inputs) -> np.ndarray:
    out, _ = run(trace=False, **inputs)
    return out
